# revision 23
# baseline (speedup 1.0000x reference)
"""Trainium2 Bass kernel for a 3-layer GCN (KnowledgeGraphGNN).

Reference (per layer i):  h = BN_i(relu(adj @ (h @ W_i) + b_i)),  then
out = h @ Wout + bout.

Sharding: nodes row-partitioned over 8 cores.  Each core keeps its adj^T
column block [N, R] resident in SBUF as fp8 (adj is 0/1 -> exact).  The
aggregation runs in transposed space with the layer weight factored OUT
of the contraction:

    T^T = S^T @ adjT_c  with S = raw activations (or x for layer 0)
    ph  = W_fold^T @ T^T (+ rank-1 degree term for BN folding)

so layer 0 needs no local stationary build at all -- the x k-tiles are
the stationary operand.  adj and x are pre-swizzled on the host into the
exact SBUF layout (per-partition-contiguous), so every streaming DMA has
large contiguous descriptors and near-zero descriptor-generation cost;
chunk-0 adjacency columns are streamed first so the PE starts after
~1/8th of the 8MB has landed.

BN is folded into the next layer algebraically (a = gamma/sigma,
r = (beta - mu a) @ W, d = adj @ 1):

    h_{i+1} = (adj @ R_i) @ diag(a_i) W  +  d x r_i

Boundary pipeline per hidden layer: chunk-0 activations AllGather while
chunk-1 aggregates; the next layer's part-A k-tiles run on the chunk-0
gather while the chunk-1 (+BN stats) gather is in flight.  Ship layout is
p-major so every gather reload is a single large DMA with 1KB-contiguous
per-partition descriptors.  Each data gather is chased by a no-op flush
gather whose input is gated by a tiny DMA so the trigger order on the
collective queue matches the pipeline.  The final layer ships only a
[128,2] f32 stat blob; its affine is folded into the output matmul
stationary (a .. Wout), producing out^T on-chip (host transposes back).
"""

import numpy as np
import ml_dtypes

BF16 = ml_dtypes.bfloat16
FP8 = ml_dtypes.float8_e4m3

N = 8192          # nodes
DH = 128          # hidden dim (= partition count)
DOUT = 64
NC = 8            # cores
R = N // NC       # rows per core = 1024
KT = N // 128     # contraction k-tiles = 64
G = 8             # k-tile groups (8 tiles each)
RT = R // 128     # node tiles per core = 8
NLAYERS = 3
EPS = 1e-5
NB = 516          # AG-b payload rows: 512 activation rows + 4 rows stats
DUMMY_ROWS = 64   # rows of the warm-up AllGather (0 = no dummy)
FLUSH_A = False   # chase AG-a with a no-op flush gather

_cache = {}


def _build_module():
    from concourse import bacc, tile
    from concourse.bass_types import AP
    import concourse.mybir as mybir

    f32 = mybir.dt.float32
    bf16 = mybir.dt.bfloat16
    fp8 = mybir.dt.float8e4
    AF = mybir.ActivationFunctionType
    ALU = mybir.AluOpType

    nc = bacc.Bacc(None, target_bir_lowering=False, num_devices=NC)

    # ---- kernel I/O --------------------------------------------------------
    # adjr: host-swizzled adj^T block, per-partition layout (h, g, s, 512)
    adjr = nc.dram_tensor("adjr", [128, 65536], fp8, kind="ExternalInput")
    # xr: host-swizzled x, per-partition layout (g, s, 128)
    xr = nc.dram_tensor("xr", [128, 8192], bf16, kind="ExternalInput")
    # wpack: w0 | w1 | w2 | wout | idn  along the free axis
    wpack = nc.dram_tensor("wpack", [128, 576], bf16, kind="ExternalInput")
    # bgb: bias | gamma | beta, [128, 3] each
    bgb = nc.dram_tensor("bgb", [128, 9], f32, kind="ExternalInput")
    # drow: degrees [1, R] | bout [1, 64]
    drow = nc.dram_tensor("drow", [1, R + DOUT], bf16, kind="ExternalInput")
    out = nc.dram_tensor("out", [DOUT, R], f32, kind="ExternalOutput")

    rg = [list(range(NC))]

    with tile.TileContext(nc) as tc:
        with (
            tc.tile_pool(name="const", bufs=1) as const,
            tc.tile_pool(name="adjp", bufs=1) as adjp,
            tc.tile_pool(name="sp", bufs=1) as sp,
            tc.tile_pool(name="work", bufs=1) as work,
            tc.tile_pool(name="psA", bufs=1, space="PSUM") as psA,
            tc.tile_pool(name="psH", bufs=1, space="PSUM") as psH,
            tc.tile_pool(name="psS", bufs=2, space="PSUM") as psS,
            tc.tile_pool(name="psT", bufs=2, space="PSUM") as psT,
            tc.tile_pool(name="dram", bufs=1, space="DRAM") as dram,
        ):
            # ---- constants (3 packed DMAs on scalar) -----------------------
            wp_sb = const.tile([128, 576], bf16, name="wp_sb")
            nc.scalar.dma_start(wp_sb[:], wpack[:])
            w_sb = [wp_sb[:, 128 * i : 128 * (i + 1)] for i in range(NLAYERS)]
            wout_sb = wp_sb[:, 384:448]
            idn_sb = wp_sb[:, 448:576]
            bgb_sb = const.tile([128, 9], f32, name="bgb_sb")
            nc.scalar.dma_start(bgb_sb[:], bgb[:])
            bias_sb = bgb_sb[:, 0:3]
            gamma_sb = bgb_sb[:, 3:6]
            beta_sb = bgb_sb[:, 6:9]
            drow_sb = const.tile([1, R + DOUT], bf16, name="drow_sb")
            nc.scalar.dma_start(drow_sb[:], drow[:])
            d_sb = drow_sb[:, 0:R]
            boutb_sb = drow_sb[:, R : R + DOUT]

            ones512 = const.tile([1, 512], bf16, name="ones512")
            nc.vector.memset(ones512[:], 1.0)
            # warm the scalar-engine activation tables (Relu/Square) so no
            # ACT_TABLE_LOAD lands mid-pipeline
            warm = const.tile([1, 4], f32, name="warm")
            nc.scalar.activation(warm[:, 0:1], ones512[:, 0:1], AF.Square)
            nc.scalar.activation(warm[:, 1:2], ones512[:, 0:1], AF.Relu)
            nc.scalar.activation(warm[:, 2:3], ones512[:, 0:1], AF.Copy,
                                 scale=1.0)
            nc.scalar.sqrt(warm[:, 3:4], warm[:, 0:1])
            w0f = const.tile([128, 128], f32, name="w0f")
            nc.scalar.activation(w0f[:], w_sb[0], AF.Copy, scale=1.0)

            # dummy AllGather triggered at program start (no deps): absorbs
            # the first-collective ncfw setup off the real collectives.
            if DUMMY_ROWS:
                agiw = dram.tile([DUMMY_ROWS, 128], bf16, name="agiw",
                                 tag="agiw")
                agow = dram.tile([NC * DUMMY_ROWS, 128], bf16, name="agow",
                                 tag="agow", addr_space="Shared")
                nc.gpsimd.collective_compute(
                    "AllGather", ALU.bypass, replica_groups=rg,
                    ins=[agiw.opt()], outs=[agow.opt()],
                )

            # ---- x k-tiles (stationary for layer 0), 2 streaming DMAs -----
            xs = const.tile([128, G, 8, 128], bf16, name="xs_sb")
            for half in range(2):
                src = xr[:, half * 4096 : (half + 1) * 4096].rearrange(
                    "p (g s c) -> p g s c", g=4, s=8
                )
                eng = nc.sync if half == 0 else nc.scalar
                eng.dma_start(xs[:, half * 4 : half * 4 + 4, :, :], src)

            # ---- adj^T streamed per (column-half, group-pair) --------------
            # chunk-0 columns (h=0) for all k-groups land first.  Source is
            # per-partition contiguous (8KB runs) thanks to the host swizzle.
            adj_sb = adjp.tile([128, G, 8, R], fp8, name="adj_sb", tag="adj_sb")
            for h in range(2):
                for j in range(4):
                    src = adjr[
                        :, h * 32768 + j * 8192 : h * 32768 + (j + 1) * 8192
                    ].rearrange("p (g s c) -> p g s c", g=2, s=8)
                    eng = nc.sync if j % 2 == 0 else nc.scalar
                    eng.dma_start(
                        adj_sb[:, 2 * j : 2 * j + 2, :, h * 512 : (h + 1) * 512],
                        src,
                    )

            def adj_mv(k, lo, size):
                g, sub = divmod(k, 8)
                return adj_sb[:, g, sub, lo : lo + size]

            # ---- gathered stationary activation tiles, double buffered ----
            s_A = [
                sp.tile([128, G, 4, 128], bf16, name=f"ssetA{st}",
                        tag=f"ssetA{st}")
                for st in range(2)
            ]
            s_B = [
                sp.tile([128, G, 4, 128], bf16, name=f"ssetB{st}",
                        tag=f"ssetB{st}")
                for st in range(2)
            ]

            def s_tile(st, k):
                g, sub = divmod(k, 8)
                if sub < 4:
                    return s_A[st][:, g, sub, :]
                return s_B[st][:, g, sub - 4, :]

            def x_tile(k):
                g, sub = divmod(k, 8)
                return xs[:, g, sub, :]

            # per-boundary DRAM comm tiles (boundaries 0->1 and 1->2)
            agiA = [
                dram.tile([512, 128], bf16, name=f"agiA{i}", tag=f"agiA{i}")
                for i in range(2)
            ]
            agoA = [
                dram.tile([NC * 512, 128], bf16, name=f"agoA{i}",
                          tag=f"agoA{i}", addr_space="Shared")
                for i in range(2)
            ]
            agiB = [
                dram.tile([NB, 128], bf16, name=f"agiB{i}", tag=f"agiB{i}")
                for i in range(2)
            ]
            agoB = [
                dram.tile([NC * NB, 128], bf16, name=f"agoB{i}",
                          tag=f"agoB{i}", addr_space="Shared")
                for i in range(2)
            ]
            # final-layer stats AllGather buffers (f32 sums bitcast as bf16)
            agi2 = dram.tile([4, 128], bf16, name="agi2", tag="agi2")
            ago2 = dram.tile([32, 128], bf16, name="ago2", tag="ago2",
                             addr_space="Shared")
            # tiny flush collectives: a collective's completion counter only
            # advances promptly when another collective follows it, so chase
            # each gather with a no-op gather.  Each flush input is gated by
            # a tiny DMA so its trigger lands right after its gather's.
            agif = [
                dram.tile([1, 128], bf16, name=f"agif{i}", tag=f"agif{i}")
                for i in range(2)
            ]
            agof = [
                dram.tile([NC, 128], bf16, name=f"agof{i}", tag=f"agof{i}",
                          addr_space="Shared")
                for i in range(2)
            ]
            agif2 = dram.tile([1, 128], bf16, name="agif2", tag="agif2")
            agof2 = dram.tile([NC, 128], bf16, name="agof2", tag="agof2",
                              addr_space="Shared")
            agifb = [
                dram.tile([1, 128], bf16, name=f"agifb{i}", tag=f"agifb{i}")
                for i in range(2)
            ]
            agofb = [
                dram.tile([NC, 128], bf16, name=f"agofb{i}", tag=f"agofb{i}",
                          addr_space="Shared")
                for i in range(2)
            ]

            gstats = [None, None]
            _wa = [None]
            _rrow = [None]

            def emit_bn_fold(i):
                # combine gathered stats of layer i-1; build Wa = diag(a) W_i
                # and r = (beta - mu a) @ W_i.
                gs = gstats[i - 1]  # [128, 16] f32: (rank, (sum, sumsq))
                gsc = work.tile([128, 8], f32, name="gsc", tag="gsc")
                st2 = work.tile([128, 2], f32, name="st2", tag="st2")
                nc.vector.tensor_add(gsc[:], gs[:, 0:8], gs[:, 8:16])
                nc.vector.tensor_add(gsc[:, 0:4], gsc[:, 0:4], gsc[:, 4:8])
                nc.vector.tensor_add(st2[:], gsc[:, 0:2], gsc[:, 2:4])
                mu = work.tile([128, 1], f32, name="mu", tag="mu")
                ex2 = work.tile([128, 1], f32, name="ex2", tag="ex2")
                var = work.tile([128, 1], f32, name="var", tag="var")
                inv = work.tile([128, 1], f32, name="inv", tag="inv")
                aco = work.tile([128, 1], f32, name="aco", tag="aco")
                cco = work.tile([128, 1], f32, name="cco", tag="cco")
                ccb = work.tile([128, 1], bf16, name="ccb", tag="ccb")
                nc.vector.tensor_scalar_mul(mu[:], st2[:, 0:1], 1.0 / N)
                nc.vector.tensor_scalar_mul(ex2[:], st2[:, 1:2], 1.0 / N)
                nc.vector.tensor_mul(var[:], mu[:], mu[:])
                nc.vector.tensor_sub(var[:], ex2[:], var[:])
                nc.vector.tensor_scalar_add(var[:], var[:], EPS)
                sd = work.tile([128, 1], f32, name="sd", tag="sd")
                nc.scalar.sqrt(sd[:], var[:])
                nc.vector.reciprocal(inv[:], sd[:])
                nc.vector.tensor_mul(aco[:], gamma_sb[:, i - 1 : i], inv[:])
                nc.vector.tensor_mul(cco[:], mu[:], aco[:])
                nc.vector.tensor_sub(cco[:], beta_sb[:, i - 1 : i], cco[:])
                nc.vector.tensor_copy(ccb[:], cco[:])
                wa = work.tile([128, 128], f32, name="wa", tag="wa")
                nc.scalar.activation(wa[:], w_sb[i], AF.Copy, scale=aco[:])
                pr = psS.tile([1, 128], f32, name="pr", tag="psS")
                nc.tensor.matmul(pr[:], ccb[:], w_sb[i])
                rrow = work.tile([1, 128], bf16, name="rrow", tag="rrow")
                nc.vector.tensor_copy(rrow[:], pr[:])
                _wa[0] = wa
                _rrow[0] = rrow

            for i in range(NLAYERS):
                st = i % 2            # gathered stationary set this layer
                stn = (i + 1) % 2     # set the boundary reload writes
                last = i == NLAYERS - 1

                zb = work.tile([128, R], bf16, name="zb", tag="zb")
                sq = work.tile([128, R], f32, name="sq", tag="sq")
                st4 = work.tile([128, 4], f32, name="st4", tag="st4")
                if not last:
                    rnat = work.tile(
                        [128, 8, 128], bf16, name=f"rnat{i % 2}",
                        tag=f"rnat{i % 2}",
                    )

                pa = [
                    psA.tile([128, 512], f32, name=f"pa{c}", tag=f"pa{c}")
                    for c in range(2)
                ]
                ph = [
                    psH.tile([128, 512], f32, name=f"ph{c}", tag=f"ph{c}")
                    for c in range(2)
                ]
                pm = work.tile([128, R], f32, name="pm", tag="pm")

                def agg(c, klist, start, stop):
                    stat = x_tile if i == 0 else (lambda k: s_tile(st, k))
                    for j, k in enumerate(klist):
                        nc.tensor.matmul(
                            pa[c][:], stat(k), adj_mv(k, c * 512, 512),
                            start=(start and j == 0),
                            stop=(stop and j == len(klist) - 1),
                        )

                def finish_chunk(c):
                    # pm copy, (BN seed +) folded-W matmul -> ph[c]
                    lo = c * 512
                    nc.vector.tensor_copy(pm[:, lo : lo + 512], pa[c][:])
                    if i == 0:
                        nc.tensor.matmul(
                            ph[c][:], w0f[:], pm[:, lo : lo + 512],
                            start=True, stop=True,
                        )
                    else:
                        nc.tensor.matmul(
                            ph[c][:], _rrow[0][:], d_sb[:, lo : lo + 512],
                            start=True, stop=False,
                        )
                        nc.tensor.matmul(
                            ph[c][:], _wa[0][:], pm[:, lo : lo + 512],
                            start=False, stop=True,
                        )

                def relu_chunk(c):
                    # scalar relu+bias (accum sum) then vector square (accum)
                    lo = c * 512
                    nc.scalar.activation(
                        zb[:, lo : lo + 512],
                        ph[c][:],
                        AF.Relu,
                        bias=bias_sb[:, i : i + 1],
                        scale=1.0,
                        accum_out=st4[:, 2 * c : 2 * c + 1],
                    )
                    nc.scalar.activation(
                        sq[:, lo : lo + 512],
                        zb[:, lo : lo + 512],
                        AF.Square,
                        accum_out=st4[:, 2 * c + 1 : 2 * c + 2],
                    )

                def transpose_chunk(c):
                    # PE transposes of 4 zb tiles into natural-layout rnat
                    for t in range(4 * c, 4 * c + 4):
                        ptp = psT.tile([128, 128], bf16, name="ptp", tag="psT")
                        nc.tensor.transpose(
                            ptp[:], zb[:, t * 128 : (t + 1) * 128], idn_sb
                        )
                        nc.vector.tensor_copy(rnat[:, t, :], ptp[:])

                def ship_chunk(c):
                    # DMA natural-layout activations to the collective input,
                    # p-major within the 512-row chunk (row = p*4 + k) so the
                    # reload is one DMA with 1KB-contiguous descriptors.
                    dst = (agiA[i] if c == 0 else agiB[i])[0:512, :]
                    nc.sync.dma_start(
                        dst.rearrange("(p k) c -> p k c", p=128),
                        rnat[:, 4 * c : 4 * c + 4, :],
                    )

                def stats_ship():
                    # combine chunk stats and DMA them to the collective input
                    # as a flat p-major blob (rows 512:516 of agiB / agi2).
                    st2o = work.tile([128, 2], f32, name="st2o", tag="st2o")
                    nc.vector.tensor_add(st2o[:], st4[:, 0:2], st4[:, 2:4])
                    if not last:
                        nc.scalar.dma_start(
                            agiB[i][512:516, :], st2o[:].bitcast(bf16)
                        )
                    else:
                        nc.scalar.dma_start(agi2[:], st2o[:].bitcast(bf16))

                def launch_a():
                    nc.gpsimd.collective_compute(
                        "AllGather", ALU.bypass, replica_groups=rg,
                        ins=[agiA[i].opt()], outs=[agoA[i].opt()],
                    )
                    if FLUSH_A:
                        # gate the flush input on the shipped chunk so its
                        # trigger queues right behind the data gather's
                        nc.sync.dma_start(agif[i][:], zb[0:1, 0:128])
                        nc.gpsimd.collective_compute(
                            "AllGather", ALU.bypass, replica_groups=rg,
                            ins=[agif[i].opt()], outs=[agof[i].opt()],
                        )

                def launch_b():
                    if not last:
                        nc.gpsimd.collective_compute(
                            "AllGather", ALU.bypass, replica_groups=rg,
                            ins=[agiB[i].opt()], outs=[agoB[i].opt()],
                        )
                    else:
                        nc.gpsimd.collective_compute(
                            "AllGather", ALU.bypass, replica_groups=rg,
                            ins=[agi2.opt()], outs=[ago2.opt()],
                        )

                def reload_a():
                    # gathered chunk-0 rows -> A half of the other set.
                    # One DMA: row g*512 + p*4 + k -> [p, g, k, c].
                    nc.scalar.dma_start(
                        s_A[stn][:],
                        agoA[i][:].rearrange(
                            "(g p k) c -> p g k c", g=G, p=128
                        ),
                    )

                def reload_b():
                    # gathered chunk-1 rows (rank blocks of NB=516 rows:
                    # 512 p-major activation rows + 4 stats rows).
                    base = agoB[i][:]
                    src = AP(
                        base.tensor,
                        base.offset,
                        [[512, 128], [NB * 128, G], [128, 4], [1, 128]],
                    )
                    nc.sync.dma_start(s_B[stn][:], src)
                    # stats blob: one DMA; p-major blob means the per-rank
                    # 512-byte tail is stride-4 per partition.
                    gst = work.tile([128, 2 * G], f32, name=f"gstats{i}",
                                    tag=f"gstats{i}")
                    srcs = AP(
                        base.tensor,
                        base.offset + 512 * 128,
                        [[4, 128], [NB * 128, G], [1, 4]],
                    )
                    nc.scalar.dma_start(
                        gst[:].bitcast(bf16).rearrange(
                            "p (g x) -> p g x", g=G
                        ),
                        srcs,
                    )
                    gstats[i] = gst

                # ================= layer body =================
                if i == 0:
                    # chunk 0: stream k-groups as adj (h=0) lands
                    agg(0, range(KT), start=True, stop=True)
                    finish_chunk(0)
                    relu_chunk(0)
                    transpose_chunk(0)
                    ship_chunk(0)
                    launch_a()
                    agg(1, range(KT), start=True, stop=True)
                    finish_chunk(1)
                    relu_chunk(1)
                    transpose_chunk(1)
                    ship_chunk(1)
                    stats_ship()
                    nc.sync.dma_start(agifb[i][:], zb[0:1, 512:640])
                    reload_a()
                    launch_b()
                    nc.gpsimd.collective_compute(
                        "AllGather", ALU.bypass, replica_groups=rg,
                        ins=[agifb[i].opt()], outs=[agofb[i].opt()],
                    )
                    reload_b()
                else:
                    ka = [g * 8 + s for g in range(G) for s in range(4)]
                    kb = [g * 8 + s for g in range(G) for s in range(4, 8)]
                    # part A (chunk-0 gather) for both output chunks
                    agg(0, ka, start=True, stop=False)
                    agg(1, ka, start=True, stop=False)
                    # part B chunk 0 runs as soon as the chunk-1 gather lands;
                    # the BN-fold (vector/scalar) overlaps it.
                    agg(0, kb, start=False, stop=True)
                    emit_bn_fold(i)
                    finish_chunk(0)
                    agg(1, kb[: len(kb) // 2], start=False, stop=False)
                    relu_chunk(0)
                    if not last:
                        transpose_chunk(0)
                        ship_chunk(0)
                        launch_a()
                    agg(1, kb[len(kb) // 2 :], start=False, stop=True)
                    finish_chunk(1)
                    relu_chunk(1)
                    if not last:
                        transpose_chunk(1)
                        ship_chunk(1)
                    stats_ship()
                    if not last:
                        nc.sync.dma_start(agifb[i][:], zb[0:1, 512:640])
                        reload_a()
                    launch_b()
                    if not last:
                        nc.gpsimd.collective_compute(
                            "AllGather", ALU.bypass, replica_groups=rg,
                            ins=[agifb[i].opt()], outs=[agofb[i].opt()],
                        )
                        reload_b()

            # ---- output: out^T = (a .. Wout)^T zb + (c Wout + bout) 1^T ---
            gs2 = work.tile([128, 2 * G], f32, name="gs2", tag="gs2")
            base2 = ago2[:]
            nc.scalar.dma_start(
                gs2[:].bitcast(bf16).rearrange("p (g x) -> p g x", g=G),
                AP(base2.tensor, base2.offset, [[4, 128], [4 * 128, G], [1, 4]]),
            )
            nc.sync.dma_start(agif2[:], zb[0:1, 512:640])
            nc.gpsimd.collective_compute(
                "AllGather", ALU.bypass, replica_groups=rg,
                ins=[agif2.opt()], outs=[agof2.opt()],
            )
            gsc2 = work.tile([128, 8], f32, name="gsc2", tag="gsc2")
            gsum = work.tile([128, 2], f32, name="gsum", tag="gsum")
            nc.vector.tensor_add(gsc2[:], gs2[:, 0:8], gs2[:, 8:16])
            nc.vector.tensor_add(gsc2[:, 0:4], gsc2[:, 0:4], gsc2[:, 4:8])
            nc.vector.tensor_add(gsum[:], gsc2[:, 0:2], gsc2[:, 2:4])
            mu2 = work.tile([128, 1], f32, name="mu2", tag="mu2")
            ex22 = work.tile([128, 1], f32, name="ex22", tag="ex22")
            var2 = work.tile([128, 1], f32, name="var2", tag="var2")
            inv2 = work.tile([128, 1], f32, name="inv2", tag="inv2")
            aco2 = work.tile([128, 1], f32, name="aco2", tag="aco2")
            cco2 = work.tile([128, 1], f32, name="cco2", tag="cco2")
            ccb2 = work.tile([128, 1], bf16, name="ccb2", tag="ccb2")
            nc.vector.tensor_scalar_mul(mu2[:], gsum[:, 0:1], 1.0 / N)
            nc.vector.tensor_scalar_mul(ex22[:], gsum[:, 1:2], 1.0 / N)
            nc.vector.tensor_mul(var2[:], mu2[:], mu2[:])
            nc.vector.tensor_sub(var2[:], ex22[:], var2[:])
            nc.vector.tensor_scalar_add(var2[:], var2[:], EPS)
            sd2 = work.tile([128, 1], f32, name="sd2", tag="sd2")
            nc.scalar.sqrt(sd2[:], var2[:])
            nc.vector.reciprocal(inv2[:], sd2[:])
            nc.vector.tensor_mul(aco2[:], gamma_sb[:, 2:3], inv2[:])
            nc.vector.tensor_mul(cco2[:], mu2[:], aco2[:])
            nc.vector.tensor_sub(cco2[:], beta_sb[:, 2:3], cco2[:])
            nc.vector.tensor_copy(ccb2[:], cco2[:])
            wscl = work.tile([128, DOUT], bf16, name="wscl", tag="wscl")
            nc.scalar.activation(wscl[:], wout_sb, AF.Copy, scale=aco2[:])
            pr2 = psS.tile([1, DOUT], f32, name="pr2", tag="psS")
            nc.tensor.matmul(pr2[:], ccb2[:], wout_sb)
            crow = work.tile([1, DOUT], f32, name="crow", tag="crow")
            nc.vector.tensor_copy(crow[:], pr2[:])
            cbrow = work.tile([1, DOUT], bf16, name="cbrow", tag="cbrow")
            nc.vector.tensor_add(cbrow[:], crow[:], boutb_sb)
            osbT = work.tile([DOUT, R], f32, name="osbT", tag="osbT")
            for c in range(2):
                lo = c * 512
                poT = psS.tile([DOUT, 512], f32, name="poT", tag="psS")
                nc.tensor.matmul(
                    poT[:], cbrow[:], ones512[:], start=True, stop=False
                )
                nc.tensor.matmul(
                    poT[:], wscl[:], zb[:, lo : lo + 512],
                    start=False, stop=True,
                )
                nc.vector.tensor_copy(osbT[:, lo : lo + 512], poT[:])
                eng = nc.sync if c == 0 else nc.scalar
                eng.dma_start(out[:, lo : lo + 512], osbT[:, lo : lo + 512])

    nc.compile()
    return nc


def _get_module():
    if "nc" not in _cache:
        _cache["nc"] = _build_module()
    return _cache["nc"]


def _prep_inputs(inputs):
    """Host-side sharding / layout prep (cast + swizzle + degrees)."""
    x = np.asarray(inputs["x"], np.float32)
    adj = np.asarray(inputs["adj"], np.float32)
    # xr[p, (g, s, c)] = x[g*1024 + s*128 + p, c]
    xr = np.ascontiguousarray(
        x.astype(BF16).reshape(8, 8, 128, 128).transpose(2, 0, 1, 3)
    ).reshape(128, 8192)
    bias = np.stack(
        [np.asarray(inputs[f"b{i}"], np.float32) for i in range(NLAYERS)],
        axis=1,
    )
    gamma = np.stack(
        [np.asarray(inputs[f"g{i}"], np.float32) for i in range(NLAYERS)],
        axis=1,
    )
    beta = np.stack(
        [np.asarray(inputs[f"be{i}"], np.float32) for i in range(NLAYERS)],
        axis=1,
    )
    wpack = np.concatenate(
        [
            np.asarray(inputs["W0"], np.float32),
            np.asarray(inputs["W1"], np.float32),
            np.asarray(inputs["W2"], np.float32),
            np.asarray(inputs["Wout"], np.float32),
            np.eye(128, dtype=np.float32),
        ],
        axis=1,
    ).astype(BF16)
    common = {
        "xr": xr,
        "wpack": wpack,
        "bgb": np.concatenate([bias, gamma, beta], axis=1),
    }
    deg = adj.sum(axis=1)                                          # [N]
    bout = np.asarray(inputs["bout"], np.float32).reshape(1, DOUT)
    in_maps = []
    for c in range(NC):
        rows = slice(c * R, (c + 1) * R)
        adjt_c = adj[rows, :].astype(FP8).T                        # [N, R]
        # adjr[p, (h, g, s, c)] = adjT[g*1024 + s*128 + p, h*512 + c]
        adjr = np.ascontiguousarray(
            adjt_c.reshape(8, 8, 128, 2, 512).transpose(2, 3, 0, 1, 4)
        ).reshape(128, 65536)
        d_c = deg[rows].reshape(1, R)
        drow_c = np.concatenate([d_c, bout], axis=1).astype(BF16)
        in_maps.append({"adjr": adjr, "drow": drow_c, **common})
    return in_maps


def run(inputs, trace=False):
    from concourse.bass_utils import run_bass_kernel_spmd

    nc = _get_module()
    in_maps = _prep_inputs(inputs)
    res = run_bass_kernel_spmd(
        nc, in_maps, core_ids=list(range(NC)), trace=trace
    )
    out = np.concatenate(
        [res.results[c]["out"].T for c in range(NC)], axis=0
    ).astype(np.float32)
    return out, res


def kernel(**inputs):
    out, _ = run(inputs, trace=False)
    return out
